# revision 21
# baseline (speedup 1.0000x reference)
"""Trainium2 Bass kernel for nn_AttentionDecoder (Bahdanau attention + GRU step
+ vocab projection w/ log-softmax), SPMD over 8 NeuronCores.

Strategy (zero collectives — on-chip AllReduce costs ~60us, measured):
  - The small H-sized chain (embed/attention/combine/GRU) is replicated on all
    8 cores in bf16; every core computes the same h_new.
  - The [50257, 1024] output projection is sharded over vocab: core c owns
    rows [c*6283, (c+1)*6283) of out_W (padded to 50264), stored transposed +
    packed host-side as fp8e4m3 scaled by 256 (values ~N(0, 0.02^2) would be
    subnormal in fp8 otherwise).
  - Each core returns its unnormalized logits shard + sum(exp(logits_shard));
    the host combines the 8 scalars into logZ and subtracts (exact math:
    log_softmax(x) = x - log(sum(exp(x)))).

Outputs match reference(): (logp [1,50257] f32, hidden [1,1,1024] f32,
attn_weights [1,512] f32).
"""
import numpy as np
import ml_dtypes

import concourse.bacc as bacc
import concourse.mybir as mybir
import concourse.tile as tile
from concourse.tile import add_dep_helper
from concourse.bass_utils import run_bass_kernel_spmd

H = 1024
L = 512
V = 50257
NCORES = 8
VC = 6283                 # vocab rows per core
VPAD = VC * NCORES        # 50264
OUT_SCALE = 256.0         # fp8 pre-scale for out_W / out_b

BF16 = mybir.dt.bfloat16
F32 = mybir.dt.float32
FP8 = mybir.dt.float8e4
NP_BF16 = ml_dtypes.bfloat16
NP_FP8 = ml_dtypes.float8_e4m3fn

ACT = mybir.ActivationFunctionType

# v-tiles of the per-core vocab shard: 12 x 512 + 1 x 139
VT_SIZES = [512] * 12 + [VC - 12 * 512]
VT_OFFS = np.cumsum([0] + VT_SIZES).tolist()

TRACE = False
DMA_CHAIN = False
LAST_EXEC_NS = None

_CACHE = {}


def _build():
    nc = bacc.Bacc("TRN2", target_bir_lowering=False, debug=False,
                   num_devices=NCORES)

    def din(name, shape, dt):
        return nc.dram_tensor(name, shape, dt, kind="ExternalInput")

    catp = din("catp", [128, 145], BF16)   # catT(16) | one(1) | ident(128)
    h0f = din("h0f", [1, H], F32)
    attnW = din("attnW", [128, 16 * 512], BF16)  # attn_W.T packed, 16 k-chunks

    encp = din("encp", [128, 4 * 1024], BF16)    # enc packed, 4 l-chunks
    combW = din("combW", [128, 16 * 1024], FP8)   # comb_W.T packed x256, 16 k-chunks

    # W_hh.T / W_ih.T packed slab-major: slab t (gate-tile) = 8 k-chunks x 512
    whh = din("whh", [128, 6 * 8 * 512], BF16)

    wih = din("wih", [128, 6 * 8 * 512], BF16)

    outW = din("outW", [128, 8 * VC], FP8)       # per-core shard.T, v-tile slabs
    cstb = din("cstb", [1, 512 + H + 6 * H + VC], BF16)  # attnb|combb|bhh|bih|outb

    logits = nc.dram_tensor("logits", [1, VC], F32, kind="ExternalOutput")
    sloc = nc.dram_tensor("sloc", [1, 1], F32, kind="ExternalOutput")
    hout = nc.dram_tensor("hout", [1, H], F32, kind="ExternalOutput")
    awout = nc.dram_tensor("awout", [1, L], F32, kind="ExternalOutput")

    with tile.TileContext(nc) as tc:
        with (
            tc.tile_pool(name="cst", bufs=1) as cst,
            tc.tile_pool(name="wsb", bufs=1) as wsb,
            tc.tile_pool(name="gruh", bufs=3) as gruh,
            tc.tile_pool(name="grui", bufs=2) as grui,
            tc.tile_pool(name="ow", bufs=8) as ow,
            tc.tile_pool(name="vec", bufs=1) as vec,
            tc.tile_pool(name="tmp", bufs=2) as tmp,
            tc.tile_pool(name="gt", bufs=1) as gt,
            tc.tile_pool(name="gt2", bufs=2) as gt2,
            tc.tile_pool(name="ps", bufs=2, space="PSUM") as ps,
            tc.tile_pool(name="psg", bufs=2, space="PSUM") as psg,
            tc.tile_pool(name="psb", bufs=4, space="PSUM") as psb,
        ):
            # ---- constants, packed into 3 DMAs
            catp_sb = cst.tile([128, 145], BF16)
            nc.sync.dma_start(out=catp_sb[:], in_=catp[:])
            catT_sb = catp_sb[:, 0:16]
            one_sb = catp_sb[:1, 16:17]
            id_sb = catp_sb[:, 17:145]
            cstb_sb = cst.tile([1, 512 + H + 6 * H + VC], BF16)
            nc.sync.dma_start(out=cstb_sb[:], in_=cstb[:])
            attnb_sb = cstb_sb[:, 0:512]
            combb_sb = cstb_sb[:, 512:512 + H]
            bhh_sb = cstb_sb[:, 512 + H:512 + H + 3 * H]
            bih_sb = cstb_sb[:, 512 + 4 * H:512 + 4 * H + 3 * H]
            outb_sb = cstb_sb[:, 512 + 7 * H:]
            h0f_sb = cst.tile([1, H], F32)
            nc.sync.dma_start(out=h0f_sb[:], in_=h0f[:])

            # ---- weights; whh first: gh = h0 @ W_hh.T runs during the attn
            # phase and doubles as the PE HAM warm-up burst. Tile spreads DMAs
            # over queues round-robin (arrival order would be ~proportional,
            # not program order), so chain them depth-2 to enforce arrival
            # order while keeping 2 transfers in flight.
            wdma = []
            attnW_sb = wsb.tile([128, 16 * 512], BF16)
            wdma.append(nc.sync.dma_start(out=attnW_sb[:], in_=attnW[:]))
            enc_sb = wsb.tile([128, 4 * 1024], BF16)
            wdma.append(nc.sync.dma_start(out=enc_sb[:], in_=encp[:]))
            whh_sl = []
            for t in range(6):
                tl = gruh.tile([128, 8 * 512], BF16, tag="whh")
                wdma.append(nc.sync.dma_start(out=tl[:],
                                              in_=whh[:, t * 4096:(t + 1) * 4096]))
                whh_sl.append(tl)
            combW_sb = wsb.tile([128, 16 * 1024], FP8)
            wdma.append(nc.sync.dma_start(out=combW_sb[:], in_=combW[:]))
            wih_sl = []
            for t in range(6):
                tl = grui.tile([128, 8 * 512], BF16, tag="wih")
                wdma.append(nc.sync.dma_start(out=tl[:],
                                              in_=wih[:, t * 4096:(t + 1) * 4096]))
                wih_sl.append(tl)
            ow_sb = []
            for t in range(13):
                nt = VT_SIZES[t]
                tl = ow.tile([128, 8 * 512], FP8, tag="ow")
                wdma.append(nc.sync.dma_start(
                    out=tl[:, :8 * nt],
                    in_=outW[:, 8 * VT_OFFS[t]:8 * (VT_OFFS[t] + nt)]))
                ow_sb.append(tl)
            if DMA_CHAIN:
                for i in range(2, len(wdma)):
                    add_dep_helper(wdma[i].ins, wdma[i - 2].ins, sync=True,
                                   reason="weight DMA arrival order")

            def gemv(psum, lhsT_cols, rhs_tiles, bias_row=None):
                n = len(rhs_tiles)
                last = n - 1 if bias_row is None else n
                for k in range(n):
                    nc.tensor.matmul(psum, lhsT=lhsT_cols[k], rhs=rhs_tiles[k],
                                     start=(k == 0), stop=(k == last))
                if bias_row is not None:
                    nc.tensor.matmul(psum, lhsT=one_sb, rhs=bias_row,
                                     start=False, stop=True)

            def transpose_row(row_bf, n_chunks, dst_bf):
                # row_bf [1, n*128] bf16 -> dst_bf [128, n] bf16 via PE transpose
                for c in range(n_chunks):
                    tp = psb.tile([128, 4], BF16, tag="po")
                    nc.tensor.transpose(tp[:, :1], row_bf[:, c * 128:(c + 1) * 128],
                                        id_sb[:1, 0:1])
                    nc.scalar.copy(dst_bf[:, c:c + 1], tp[:, :1])

            # startup HAM warm-up while attnW streams in (ident is tiny+early)
            for _ in range(64):
                p = psb.tile([1, 512], F32, tag="po")
                nc.tensor.matmul(p[:, :128], lhsT=catT_sb[:, 0:1],
                                 rhs=id_sb, start=True, stop=True)

            # ---- attention scores + softmax (first on ACT: exp)
            z1 = ps.tile([1, 512], F32, tag="pch")
            gemv(z1[:], [catT_sb[:, k:k + 1] for k in range(16)],
                 [attnW_sb[:, k * 512:(k + 1) * 512] for k in range(16)],
                 attnb_sb)
            aw_f = vec.tile([1, L], F32)
            s_att = vec.tile([1, 1], F32)
            e_att = vec.tile([1, L], F32)
            nc.scalar.activation(e_att[:], z1[:], ACT.Exp, accum_out=s_att[:])
            rs_att = vec.tile([1, 1], F32)
            nc.vector.reciprocal(rs_att[:], s_att[:])
            nc.vector.tensor_scalar_mul(aw_f[:], e_att[:], rs_att[:])
            nc.sync.dma_start(out=awout[:], in_=aw_f[:])
            aw_bf = vec.tile([1, L], BF16)
            nc.vector.tensor_copy(aw_bf[:], aw_f[:])
            awT = vec.tile([128, 4], BF16)
            transpose_row(aw_bf, 4, awT)

            # ---- attn_applied = attn_w @ enc   [1, 1024]
            aa_bf = vec.tile([1, H], BF16)
            for n in range(2):
                p = ps.tile([1, 512], F32, tag="pch")
                gemv(p[:], [awT[:, lc:lc + 1] for lc in range(4)],
                     [enc_sb[:, lc * 1024 + n * 512:lc * 1024 + (n + 1) * 512]
                      for lc in range(4)])
                nc.vector.tensor_copy(aa_bf[:, n * 512:(n + 1) * 512], p[:])
            aaT = vec.tile([128, 8], BF16)
            transpose_row(aa_bf, 8, aaT)

            # ---- gh = h0 @ W_hh.T + b_hh: fills the comb/wih DMA-wait window
            gh_sb = vec.tile([1, 3 * H], F32)
            for t in range(6):
                p = psg.tile([1, 512], F32, tag="pg")
                gemv(p[:], [catT_sb[:, 8 + k:8 + k + 1] for k in range(8)],
                     [whh_sl[t][:, k * 512:(k + 1) * 512] for k in range(8)],
                     bhh_sb[:, t * 512:(t + 1) * 512])
                nc.vector.tensor_copy(gh_sb[:, t * 512:(t + 1) * 512], p[:])

            # ---- x = relu([emb | attn_applied] @ comb_W.T + comb_b)
            x_bf = vec.tile([1, H], BF16)
            cat2_cols = [catT_sb[:, k:k + 1] for k in range(8)] + \
                        [aaT[:, k:k + 1] for k in range(8)]
            for n in range(2):
                p = ps.tile([1, 512], F32, tag="pch")
                gemv(p[:], cat2_cols,
                     [combW_sb[:, k * 1024 + n * 512:k * 1024 + (n + 1) * 512]
                      for k in range(16)],
                     combb_sb[:, n * 512:(n + 1) * 512])
                nc.scalar.activation(x_bf[:, n * 512:(n + 1) * 512], p[:], ACT.Relu,
                                     scale=1.0 / OUT_SCALE)
            xT = vec.tile([128, 8], BF16)
            transpose_row(x_bf, 8, xT)

            # ---- gi tiles (paced by wih arrival); gates right behind each
            h_f = vec.tile([1, H], F32)
            gin_sb = vec.tile([1, H], F32)
            rz_sb = vec.tile([1, 4 * 512], F32)   # r0 r1 z0 z1
            for t in range(6):
                p = psg.tile([1, 512], F32, tag="pg")
                gemv(p[:], [xT[:, k:k + 1] for k in range(8)],
                     [wih_sl[t][:, k * 512:(k + 1) * 512] for k in range(8)],
                     bih_sb[:, t * 512:(t + 1) * 512])
                for _ in range(3):
                    pw = psb.tile([1, 512], F32, tag="po")
                    nc.tensor.matmul(pw[:, :128], lhsT=catT_sb[:, 0:1],
                                     rhs=id_sb, start=True, stop=True)
                if t < 4:  # r / z gates: sigmoid(gi + gh)
                    t0 = gt2.tile([1, 512], F32, tag="gtmp")
                    nc.vector.tensor_add(t0[:], p[:], gh_sb[:, t * 512:(t + 1) * 512])
                    nc.scalar.activation(rz_sb[:, t * 512:(t + 1) * 512], t0[:],
                                         ACT.Sigmoid)
                else:  # stage gi_n to SBUF; n/h computed 1024-wide after the loop
                    nc.vector.tensor_copy(gin_sb[:, (t - 4) * 512:(t - 3) * 512],
                                          p[:])
            # n = tanh(gi_n + r * gh_n); h = n + z*(h0-n)   (full-width ops)
            t1 = gt.tile([1, H], F32, tag="ga")
            nc.vector.tensor_mul(t1[:], rz_sb[:, 0:H], gh_sb[:, 2 * H:3 * H])
            t2 = gt.tile([1, H], F32, tag="gb")
            nc.vector.tensor_add(t2[:], t1[:], gin_sb[:])
            n_sb = gt.tile([1, H], F32, tag="gn")
            nc.scalar.activation(n_sb[:], t2[:], ACT.Tanh)
            d_sb = gt.tile([1, H], F32, tag="ga")
            nc.vector.tensor_sub(d_sb[:], h0f_sb[:], n_sb[:])
            dz_sb = gt.tile([1, H], F32, tag="gb")
            nc.vector.tensor_mul(dz_sb[:], rz_sb[:, H:2 * H], d_sb[:])
            nc.vector.tensor_add(h_f[:], n_sb[:], dz_sb[:])
            nc.sync.dma_start(out=hout[:], in_=h_f[:])
            h_bf = vec.tile([1, H], BF16)
            nc.vector.tensor_copy(h_bf[:], h_f[:])
            hT = vec.tile([128, 8], BF16)
            transpose_row(h_bf, 8, hT)

            # ---- out projection shard: logits = h_new @ outW_shard.T + out_b
            sums = vec.tile([1, 16], F32)
            for t in range(13):
                nt = VT_SIZES[t]
                p = psb.tile([1, 512], F32, tag="po")
                gemv(p[:, :nt], [hT[:, k:k + 1] for k in range(8)],
                     [ow_sb[t][:, k * nt:(k + 1) * nt] for k in range(8)],
                     outb_sb[:, VT_OFFS[t]:VT_OFFS[t] + nt])
                esc = tmp.tile([1, 512], F32, tag="esc")
                nc.scalar.activation(esc[:, :nt], p[:, :nt], ACT.Exp,
                                     scale=1.0 / OUT_SCALE,
                                     accum_out=sums[:, t:t + 1])
                lgo = tmp.tile([1, 512], F32, tag="lgo")
                nc.vector.tensor_scalar_mul(lgo[:, :nt], p[:, :nt], 1.0 / OUT_SCALE)
                nc.sync.dma_start(out=logits[:, VT_OFFS[t]:VT_OFFS[t] + nt],
                                  in_=lgo[:, :nt])
            s_sb = vec.tile([1, 1], F32)
            nc.vector.reduce_sum(s_sb[:], sums[:, :13], axis=mybir.AxisListType.X)
            nc.sync.dma_start(out=sloc[:], in_=s_sb[:])

    nc.compile()
    return nc


def _pack_chunks(mat_t, n_chunks, dtype):
    # mat_t [n_chunks*128, N] -> [128, n_chunks*N] (k-chunk c at [:, c*N:(c+1)*N])
    k, n = mat_t.shape
    assert k == n_chunks * 128
    return np.ascontiguousarray(
        mat_t.reshape(n_chunks, 128, n).transpose(1, 0, 2).reshape(128, n_chunks * n)
    ).astype(dtype)


def _pack_slabs(mat_t, tile_sizes, dtype):
    # mat_t [8*128, N] -> [128, 8*N] slab-major: slab t holds its 8 k-chunks
    # contiguously ([:, k*nt:(k+1)*nt] within the slab).
    k, n = mat_t.shape
    kc = k // 128
    mat_t = np.asarray(mat_t, dtype=dtype)
    slabs = []
    off = 0
    for nt in tile_sizes:
        blk = mat_t[:, off:off + nt]
        slabs.append(np.ascontiguousarray(
            blk.reshape(kc, 128, nt).transpose(1, 0, 2).reshape(128, kc * nt)))
        off += nt
    return np.concatenate(slabs, axis=1)


def _pack_host(input_token, hidden, encoder_outputs, emb, attn_W, attn_b,
               comb_W, comb_b, W_ih, W_hh, b_ih, b_hh, out_W, out_b):
    tok = int(np.asarray(input_token).reshape(-1)[0])
    emb = np.asarray(emb, dtype=np.float32)
    embed = emb[tok]                                   # [H]
    h0 = np.asarray(hidden, dtype=np.float32).reshape(1, H)
    cat = np.concatenate([embed[None, :], h0], axis=1)  # [1, 2H]

    catp = np.zeros((128, 145), dtype=NP_BF16)
    catp[:, 0:16] = np.ascontiguousarray(cat.reshape(16, 128).T).astype(NP_BF16)
    catp[0, 16] = 1.0
    catp[:, 17:145] = np.eye(128, dtype=NP_BF16)
    common = {
        "catp": catp,
        "h0f": h0,
        "attnW": _pack_chunks(np.asarray(attn_W, np.float32).T, 16, NP_BF16),

        "encp": _pack_chunks(np.asarray(encoder_outputs, np.float32), 4, NP_BF16),
        "combW": _pack_chunks(np.asarray(comb_W, np.float32).T * OUT_SCALE,
                               16, NP_FP8),

        "whh": _pack_slabs(np.asarray(W_hh, np.float32).T.astype(NP_BF16),
                           [512] * 6, NP_BF16),

        "wih": _pack_slabs(np.asarray(W_ih, np.float32).T.astype(NP_BF16),
                           [512] * 6, NP_BF16),

    }

    out_W = np.asarray(out_W, dtype=np.float32)
    out_b = np.asarray(out_b, dtype=np.float32)
    Wp = np.zeros((VPAD, H), dtype=np.float32)
    Wp[:V] = out_W
    bp = np.full((VPAD,), -30000.0, dtype=np.float32)
    bp[:V] = out_b

    in_maps = []
    for c in range(NCORES):
        Ws = Wp[c * VC:(c + 1) * VC]                   # [VC, H]
        Wt8 = (Ws.T * OUT_SCALE).astype(NP_FP8)        # [H, VC]
        m = dict(common)
        m["outW"] = _pack_slabs(Wt8, VT_SIZES, NP_FP8)
        cb = np.concatenate([
            np.asarray(attn_b, np.float32).reshape(-1),
            np.asarray(comb_b, np.float32).reshape(-1) * OUT_SCALE,
            np.asarray(b_hh, np.float32).reshape(-1),
            np.asarray(b_ih, np.float32).reshape(-1),
            bp[c * VC:(c + 1) * VC] * OUT_SCALE,
        ]).reshape(1, -1).astype(NP_BF16)
        m["cstb"] = cb
        in_maps.append(m)
    return in_maps


def kernel(**inputs):
    global LAST_EXEC_NS
    if "nc" not in _CACHE:
        _CACHE["nc"] = _build()
    nc = _CACHE["nc"]
    in_maps = _pack_host(**inputs)
    res = run_bass_kernel_spmd(nc, in_maps, list(range(NCORES)), trace=TRACE)
    LAST_EXEC_NS = res.exec_time_ns

    logits = np.concatenate([res.results[c]["logits"][0] for c in range(NCORES)])
    s_tot = float(sum(res.results[c]["sloc"][0, 0] for c in range(NCORES)))
    logp = (logits[:V] - np.log(s_tot)).reshape(1, V).astype(np.float32)
    h_new = res.results[0]["hout"].reshape(1, 1, H).astype(np.float32)
    attn_w = res.results[0]["awout"].reshape(1, L).astype(np.float32)
    return logp, h_new, attn_w


# revision 24
# speedup vs baseline: 1.0994x; 1.0994x over previous
"""Trainium2 Bass kernel for nn_AttentionDecoder (Bahdanau attention + GRU step
+ vocab projection w/ log-softmax), SPMD over 8 NeuronCores.

Strategy (zero collectives — on-chip AllReduce costs ~60us, measured):
  - The small H-sized chain (embed/attention/combine/GRU) is replicated on all
    8 cores in bf16; every core computes the same h_new.
  - The [50257, 1024] output projection is sharded over vocab: core c owns
    rows [c*6283, (c+1)*6283) of out_W (padded to 50264), stored transposed +
    packed host-side as fp8e4m3 scaled by 256 (values ~N(0, 0.02^2) would be
    subnormal in fp8 otherwise).
  - Each core returns its unnormalized logits shard + sum(exp(logits_shard));
    the host combines the 8 scalars into logZ and subtracts (exact math:
    log_softmax(x) = x - log(sum(exp(x)))).

Outputs match reference(): (logp [1,50257] f32, hidden [1,1,1024] f32,
attn_weights [1,512] f32).
"""
import numpy as np
import ml_dtypes

import concourse.bacc as bacc
import concourse.mybir as mybir
import concourse.tile as tile
from concourse.tile import add_dep_helper
from concourse.bass_utils import run_bass_kernel_spmd

H = 1024
L = 512
V = 50257
NCORES = 8
VC = 6283                 # vocab rows per core
VPAD = VC * NCORES        # 50264
OUT_SCALE = 256.0         # fp8 pre-scale for out_W / out_b

BF16 = mybir.dt.bfloat16
F32 = mybir.dt.float32
FP8 = mybir.dt.float8e4
NP_BF16 = ml_dtypes.bfloat16
NP_FP8 = ml_dtypes.float8_e4m3fn

ACT = mybir.ActivationFunctionType

# v-tiles of the per-core vocab shard: 12 x 512 + 1 x 139
VT_SIZES = [512] * 12 + [VC - 12 * 512]
VT_OFFS = np.cumsum([0] + VT_SIZES).tolist()

TRACE = False
DMA_CHAIN = False
LAST_EXEC_NS = None

_CACHE = {}


def _build():
    nc = bacc.Bacc("TRN2", target_bir_lowering=False, debug=False,
                   num_devices=NCORES)

    def din(name, shape, dt):
        return nc.dram_tensor(name, shape, dt, kind="ExternalInput")

    catp = din("catp", [128, 145], BF16)   # catT(16) | one(1) | ident(128)
    h0f = din("h0f", [1, H], F32)
    attnW = din("attnW", [128, 16 * 512], BF16)  # attn_W.T packed, 16 k-chunks

    encp = din("encp", [128, 4 * 1024], FP8)     # enc packed x256, 4 l-chunks
    combW = din("combW", [128, 16 * 1024], FP8)   # comb_W.T packed x256, 16 k-chunks

    # W_hh.T / W_ih.T packed slab-major: slab t (gate-tile) = 8 k-chunks x 512
    whh = din("whh", [128, 6 * 8 * 512], BF16)

    wih = din("wih", [128, 6 * 8 * 512], BF16)

    outW = din("outW", [128, 8 * VC], FP8)       # per-core shard.T, v-tile slabs
    cstb = din("cstb", [1, 512 + H + 6 * H + VC], BF16)  # attnb|combb|bhh|bih|outb

    logits = nc.dram_tensor("logits", [1, VC], F32, kind="ExternalOutput")
    sloc = nc.dram_tensor("sloc", [1, 1], F32, kind="ExternalOutput")
    hout = nc.dram_tensor("hout", [1, H], F32, kind="ExternalOutput")
    awout = nc.dram_tensor("awout", [1, L], F32, kind="ExternalOutput")

    with tile.TileContext(nc) as tc:
        with (
            tc.tile_pool(name="cst", bufs=1) as cst,
            tc.tile_pool(name="wsb", bufs=1) as wsb,
            tc.tile_pool(name="gruh", bufs=3) as gruh,
            tc.tile_pool(name="grui", bufs=3) as grui,
            tc.tile_pool(name="ow", bufs=7) as ow,
            tc.tile_pool(name="vec", bufs=1) as vec,
            tc.tile_pool(name="tmp", bufs=2) as tmp,
            tc.tile_pool(name="gt", bufs=1) as gt,
            tc.tile_pool(name="gt2", bufs=1) as gt2,
            tc.tile_pool(name="ps", bufs=2, space="PSUM") as ps,
            tc.tile_pool(name="psg", bufs=2, space="PSUM") as psg,
            tc.tile_pool(name="psb", bufs=4, space="PSUM") as psb,
        ):
            # ---- constants, packed into 3 DMAs
            catp_sb = cst.tile([128, 145], BF16)
            nc.sync.dma_start(out=catp_sb[:], in_=catp[:])
            catT_sb = catp_sb[:, 0:16]
            one_sb = catp_sb[:1, 16:17]
            id_sb = catp_sb[:, 17:145]
            cstb_sb = cst.tile([1, 512 + H + 6 * H + VC], BF16)
            nc.sync.dma_start(out=cstb_sb[:], in_=cstb[:])
            attnb_sb = cstb_sb[:, 0:512]
            combb_sb = cstb_sb[:, 512:512 + H]
            bhh_sb = cstb_sb[:, 512 + H:512 + H + 3 * H]
            bih_sb = cstb_sb[:, 512 + 4 * H:512 + 4 * H + 3 * H]
            outb_sb = cstb_sb[:, 512 + 7 * H:]
            h0f_sb = cst.tile([1, H], F32)
            nc.sync.dma_start(out=h0f_sb[:], in_=h0f[:])

            # ---- weights; whh first: gh = h0 @ W_hh.T runs during the attn
            # phase and doubles as the PE HAM warm-up burst. Tile spreads DMAs
            # over queues round-robin (arrival order would be ~proportional,
            # not program order), so chain them depth-2 to enforce arrival
            # order while keeping 2 transfers in flight.
            wdma = []
            attnW_sb = wsb.tile([128, 16 * 512], BF16)
            wdma.append(nc.sync.dma_start(out=attnW_sb[:], in_=attnW[:]))
            enc_sb = wsb.tile([128, 4 * 1024], FP8)
            wdma.append(nc.sync.dma_start(out=enc_sb[:], in_=encp[:]))
            combW_sb = wsb.tile([128, 16 * 1024], FP8)
            wdma.append(nc.sync.dma_start(out=combW_sb[:], in_=combW[:]))
            wih_sl = []
            for t in range(6):
                tl = grui.tile([128, 8 * 512], BF16, tag="wih")
                wdma.append(nc.sync.dma_start(out=tl[:],
                                              in_=wih[:, t * 4096:(t + 1) * 4096]))
                wih_sl.append(tl)
            whh_sl = []
            for t in range(6):
                tl = gruh.tile([128, 8 * 512], BF16, tag="whh")
                wdma.append(nc.sync.dma_start(out=tl[:],
                                              in_=whh[:, t * 4096:(t + 1) * 4096]))
                whh_sl.append(tl)
            ow_sb = []
            for t in range(13):
                nt = VT_SIZES[t]
                tl = ow.tile([128, 8 * 512], FP8, tag="ow")
                wdma.append(nc.sync.dma_start(
                    out=tl[:, :8 * nt],
                    in_=outW[:, 8 * VT_OFFS[t]:8 * (VT_OFFS[t] + nt)]))
                ow_sb.append(tl)
            if DMA_CHAIN:
                for i in range(2, len(wdma)):
                    add_dep_helper(wdma[i].ins, wdma[i - 2].ins, sync=True,
                                   reason="weight DMA arrival order")

            def gemv(psum, lhsT_cols, rhs_tiles, bias_row=None):
                n = len(rhs_tiles)
                last = n - 1 if bias_row is None else n
                for k in range(n):
                    nc.tensor.matmul(psum, lhsT=lhsT_cols[k], rhs=rhs_tiles[k],
                                     start=(k == 0), stop=(k == last))
                if bias_row is not None:
                    nc.tensor.matmul(psum, lhsT=one_sb, rhs=bias_row,
                                     start=False, stop=True)

            def transpose_row(row_bf, n_chunks, dst_bf):
                # row_bf [1, n*128] bf16 -> dst_bf [128, n] bf16 via PE transpose
                for c in range(n_chunks):
                    tp = psb.tile([128, 4], BF16, tag="po")
                    nc.tensor.transpose(tp[:, :1], row_bf[:, c * 128:(c + 1) * 128],
                                        id_sb[:1, 0:1])
                    nc.scalar.copy(dst_bf[:, c:c + 1], tp[:, :1])

            # startup HAM warm-up while attnW streams in (ident is tiny+early)
            for _ in range(64):
                p = psb.tile([1, 512], F32, tag="po")
                nc.tensor.matmul(p[:, :128], lhsT=catT_sb[:, 0:1],
                                 rhs=id_sb, start=True, stop=True)

            # ---- attention scores + softmax (first on ACT: exp)
            z1 = ps.tile([1, 512], F32, tag="pch")
            gemv(z1[:], [catT_sb[:, k:k + 1] for k in range(16)],
                 [attnW_sb[:, k * 512:(k + 1) * 512] for k in range(16)],
                 attnb_sb)
            aw_f = vec.tile([1, L], F32)
            s_att = vec.tile([1, 1], F32)
            e_att = vec.tile([1, L], F32)
            nc.scalar.activation(e_att[:], z1[:], ACT.Exp, accum_out=s_att[:])
            rs_att = vec.tile([1, 1], F32)
            nc.vector.reciprocal(rs_att[:], s_att[:])
            nc.vector.tensor_scalar_mul(aw_f[:], e_att[:], rs_att[:])
            nc.sync.dma_start(out=awout[:], in_=aw_f[:])
            aw_bf = vec.tile([1, L], BF16)
            nc.vector.tensor_copy(aw_bf[:], aw_f[:])
            awT = vec.tile([128, 4], BF16)
            transpose_row(aw_bf, 4, awT)

            # ---- attn_applied = attn_w @ enc   [1, 1024]
            aa_bf = vec.tile([1, H], BF16)
            for n in range(2):
                p = ps.tile([1, 512], F32, tag="pch")
                gemv(p[:], [awT[:, lc:lc + 1] for lc in range(4)],
                     [enc_sb[:, lc * 1024 + n * 512:lc * 1024 + (n + 1) * 512]
                      for lc in range(4)])
                nc.vector.tensor_scalar_mul(aa_bf[:, n * 512:(n + 1) * 512], p[:],
                                            1.0 / 16.0)
            aaT = vec.tile([128, 8], BF16)
            transpose_row(aa_bf, 8, aaT)

            # ---- x = relu([emb | attn_applied] @ comb_W.T + comb_b)
            x_bf = vec.tile([1, H], BF16)
            cat2_cols = [catT_sb[:, k:k + 1] for k in range(8)] + \
                        [aaT[:, k:k + 1] for k in range(8)]
            for n in range(2):
                p = ps.tile([1, 512], F32, tag="pch")
                gemv(p[:], cat2_cols,
                     [combW_sb[:, k * 1024 + n * 512:k * 1024 + (n + 1) * 512]
                      for k in range(16)],
                     combb_sb[:, n * 512:(n + 1) * 512])
                nc.scalar.activation(x_bf[:, n * 512:(n + 1) * 512], p[:], ACT.Relu,
                                     scale=1.0 / OUT_SCALE)
            xT = vec.tile([128, 8], BF16)
            transpose_row(x_bf, 8, xT)

            # ---- gi tiles (early, paced by wih) -> SBUF
            gisb = vec.tile([1, 3 * H], F32)
            for t in range(6):
                p = psg.tile([1, 512], F32, tag="pg")
                gemv(p[:], [xT[:, k:k + 1] for k in range(8)],
                     [wih_sl[t][:, k * 512:(k + 1) * 512] for k in range(8)],
                     bih_sb[:, t * 512:(t + 1) * 512])
                nc.vector.tensor_copy(gisb[:, t * 512:(t + 1) * 512], p[:])

            # ---- gh tiles (paced by whh); gates consume them from PSUM
            h_f = vec.tile([1, H], F32)
            ghn_sb = vec.tile([1, H], F32)
            rz_sb = vec.tile([1, 4 * 512], F32)   # r0 r1 z0 z1
            for t in range(6):
                p = psg.tile([1, 512], F32, tag="pg")
                gemv(p[:], [catT_sb[:, 8 + k:8 + k + 1] for k in range(8)],
                     [whh_sl[t][:, k * 512:(k + 1) * 512] for k in range(8)],
                     bhh_sb[:, t * 512:(t + 1) * 512])
                if t < 4:  # r / z gates: sigmoid(gi + gh)
                    t0 = gt2.tile([1, 512], F32, tag="gtmp")
                    nc.vector.tensor_add(t0[:], p[:], gisb[:, t * 512:(t + 1) * 512])
                    nc.scalar.activation(rz_sb[:, t * 512:(t + 1) * 512], t0[:],
                                         ACT.Sigmoid)
                else:
                    nc.vector.tensor_copy(ghn_sb[:, (t - 4) * 512:(t - 3) * 512],
                                          p[:])
            # n = tanh(gi_n + r * gh_n); h = n + z*(h0-n)   (full-width ops)
            t1 = gt.tile([1, H], F32, tag="ga")
            nc.vector.tensor_mul(t1[:], rz_sb[:, 0:H], ghn_sb[:])
            t2 = gt.tile([1, H], F32, tag="gb")
            nc.vector.tensor_add(t2[:], t1[:], gisb[:, 2 * H:3 * H])
            n_sb = gt.tile([1, H], F32, tag="gn")
            nc.scalar.activation(n_sb[:], t2[:], ACT.Tanh)
            d_sb = gt.tile([1, H], F32, tag="ga")
            nc.vector.tensor_sub(d_sb[:], h0f_sb[:], n_sb[:])
            dz_sb = gt.tile([1, H], F32, tag="gb")
            nc.vector.tensor_mul(dz_sb[:], rz_sb[:, H:2 * H], d_sb[:])
            nc.vector.tensor_add(h_f[:], n_sb[:], dz_sb[:])
            nc.sync.dma_start(out=hout[:], in_=h_f[:])
            h_bf = vec.tile([1, H], BF16)
            nc.vector.tensor_copy(h_bf[:], h_f[:])
            hT = vec.tile([128, 8], BF16)
            transpose_row(h_bf, 8, hT)

            # ---- out projection shard: logits = h_new @ outW_shard.T + out_b
            sums = vec.tile([1, 16], F32)
            for t in range(13):
                nt = VT_SIZES[t]
                p = psb.tile([1, 512], F32, tag="po")
                gemv(p[:, :nt], [hT[:, k:k + 1] for k in range(8)],
                     [ow_sb[t][:, k * nt:(k + 1) * nt] for k in range(8)],
                     outb_sb[:, VT_OFFS[t]:VT_OFFS[t] + nt])
                esc = tmp.tile([1, 512], F32, tag="eo")
                nc.scalar.activation(esc[:, :nt], p[:, :nt], ACT.Exp,
                                     scale=1.0 / OUT_SCALE,
                                     accum_out=sums[:, t:t + 1])
                lgo = tmp.tile([1, 512], F32, tag="eo")
                nc.vector.tensor_scalar_mul(lgo[:, :nt], p[:, :nt], 1.0 / OUT_SCALE)
                nc.sync.dma_start(out=logits[:, VT_OFFS[t]:VT_OFFS[t] + nt],
                                  in_=lgo[:, :nt])
            s_sb = vec.tile([1, 1], F32)
            nc.vector.reduce_sum(s_sb[:], sums[:, :13], axis=mybir.AxisListType.X)
            nc.sync.dma_start(out=sloc[:], in_=s_sb[:])

    nc.compile()
    return nc


def _pack_chunks(mat_t, n_chunks, dtype):
    # mat_t [n_chunks*128, N] -> [128, n_chunks*N] (k-chunk c at [:, c*N:(c+1)*N])
    k, n = mat_t.shape
    assert k == n_chunks * 128
    return np.ascontiguousarray(
        mat_t.reshape(n_chunks, 128, n).transpose(1, 0, 2).reshape(128, n_chunks * n)
    ).astype(dtype)


def _pack_slabs(mat_t, tile_sizes, dtype):
    # mat_t [8*128, N] -> [128, 8*N] slab-major: slab t holds its 8 k-chunks
    # contiguously ([:, k*nt:(k+1)*nt] within the slab).
    k, n = mat_t.shape
    kc = k // 128
    mat_t = np.asarray(mat_t, dtype=dtype)
    slabs = []
    off = 0
    for nt in tile_sizes:
        blk = mat_t[:, off:off + nt]
        slabs.append(np.ascontiguousarray(
            blk.reshape(kc, 128, nt).transpose(1, 0, 2).reshape(128, kc * nt)))
        off += nt
    return np.concatenate(slabs, axis=1)


def _pack_host(input_token, hidden, encoder_outputs, emb, attn_W, attn_b,
               comb_W, comb_b, W_ih, W_hh, b_ih, b_hh, out_W, out_b):
    tok = int(np.asarray(input_token).reshape(-1)[0])
    emb = np.asarray(emb, dtype=np.float32)
    embed = emb[tok]                                   # [H]
    h0 = np.asarray(hidden, dtype=np.float32).reshape(1, H)
    cat = np.concatenate([embed[None, :], h0], axis=1)  # [1, 2H]

    catp = np.zeros((128, 145), dtype=NP_BF16)
    catp[:, 0:16] = np.ascontiguousarray(cat.reshape(16, 128).T).astype(NP_BF16)
    catp[0, 16] = 1.0
    catp[:, 17:145] = np.eye(128, dtype=NP_BF16)
    common = {
        "catp": catp,
        "h0f": h0,
        "attnW": _pack_chunks(np.asarray(attn_W, np.float32).T, 16, NP_BF16),

        "encp": _pack_chunks(np.asarray(encoder_outputs, np.float32) * 16.0,
                             4, NP_FP8),
        "combW": _pack_chunks(np.asarray(comb_W, np.float32).T * OUT_SCALE,
                               16, NP_FP8),

        "whh": _pack_slabs(np.asarray(W_hh, np.float32).T.astype(NP_BF16),
                           [512] * 6, NP_BF16),

        "wih": _pack_slabs(np.asarray(W_ih, np.float32).T.astype(NP_BF16),
                           [512] * 6, NP_BF16),

    }

    out_W = np.asarray(out_W, dtype=np.float32)
    out_b = np.asarray(out_b, dtype=np.float32)
    Wp = np.zeros((VPAD, H), dtype=np.float32)
    Wp[:V] = out_W
    bp = np.full((VPAD,), -30000.0, dtype=np.float32)
    bp[:V] = out_b

    in_maps = []
    for c in range(NCORES):
        Ws = Wp[c * VC:(c + 1) * VC]                   # [VC, H]
        Wt8 = (Ws.T * OUT_SCALE).astype(NP_FP8)        # [H, VC]
        m = dict(common)
        m["outW"] = _pack_slabs(Wt8, VT_SIZES, NP_FP8)
        cb = np.concatenate([
            np.asarray(attn_b, np.float32).reshape(-1),
            np.asarray(comb_b, np.float32).reshape(-1) * OUT_SCALE,
            np.asarray(b_hh, np.float32).reshape(-1),
            np.asarray(b_ih, np.float32).reshape(-1),
            bp[c * VC:(c + 1) * VC] * OUT_SCALE,
        ]).reshape(1, -1).astype(NP_BF16)
        m["cstb"] = cb
        in_maps.append(m)
    return in_maps


def kernel(**inputs):
    global LAST_EXEC_NS
    if "nc" not in _CACHE:
        _CACHE["nc"] = _build()
    nc = _CACHE["nc"]
    in_maps = _pack_host(**inputs)
    res = run_bass_kernel_spmd(nc, in_maps, list(range(NCORES)), trace=TRACE)
    LAST_EXEC_NS = res.exec_time_ns

    logits = np.concatenate([res.results[c]["logits"][0] for c in range(NCORES)])
    s_tot = float(sum(res.results[c]["sloc"][0, 0] for c in range(NCORES)))
    logp = (logits[:V] - np.log(s_tot)).reshape(1, V).astype(np.float32)
    h_new = res.results[0]["hout"].reshape(1, 1, H).astype(np.float32)
    attn_w = res.results[0]["awout"].reshape(1, L).astype(np.float32)
    return logp, h_new, attn_w
